# revision 6
# baseline (speedup 1.0000x reference)
"""DiffuseRouter kernel for 8 TRN2 NeuronCores.

Reference computation (enable_time=False, soft_time_routing=True):
    out[b, l, d] = (1/3) * sum_g sum_e expert_emb_g[e, b, l, d]
i.e. a uniform-weighted sum of 28 expert planes per batch element.

Sharding: pure data-parallel over batch B=8 -> one batch element per core.
Each core reads its 28 expert planes (36.7 MB), reduces them on-chip,
scales by 1/3, and writes its [256, 1280] output.  No collectives needed
(B == n_cores).

v4 -- dodge SDMA engine 15.  Perfetto traces show DMA engine 15 runs
~20% slower than engines 0-14 on every run (109-111 us busy vs ~91 us
for the same per-engine byte count).  Because a plane's completion
semaphore fires only when the slowest engine finishes its partitions,
the whole pipeline paces at engine 15's rate (~3.7 us/plane instead of
~3.1).  Engine k serves fixed SBUF partitions; engine 15 owns
{92..95, 124..127}.  So each plane is host-packed to [120, 2731] f32
(40 pad zeros) and loaded into SBUF partitions 0..91 and 96..123 only:
every other engine gets exactly 8 partitions per plane, engine 15 gets
none, and the 15-way split is perfectly balanced.

The reduction is a plain DVE scalar_tensor_tensor chain in exact fp32
(one [124, 2731] op per plane, ~3.0 us, under the ~3.3 us/plane arrival
rate; SBUF partitions 92-95 are never written and their lanes' garbage
is never stored).  The 1/3 scale is folded into every add.  The last
plane is loaded as four column-chunks so the final add -> store per
chunk fires as soon as its own ~0.3 MB lands, keeping the tail to ~3 us.
"""

import numpy as np

import concourse.bacc as bacc
import concourse.tile as tile
from concourse import mybir
from concourse.alu_op_type import AluOpType
from concourse.bass_utils import run_bass_kernel_spmd

N_CORES = 8
E_TOTAL = 28  # 4 + 8 + 16 experts across the 3 granularity levels
L, D = 256, 1280
PLANE = L * D  # 327680 f32 per expert plane
NP_USED = 120  # SBUF partitions used (skip engine 15's 8)
W = -(-PLANE // NP_USED)  # 2731 f32 per partition (40 pad zeros total)
PT = 124  # tile partition extent: rows 0..91 and 96..123 are live
SCALE = 1.0 / 3.0

# Column chunking of the last plane (tail fine-graining).
N_CHUNK = 4
CW = [W // N_CHUNK + (1 if i < W % N_CHUNK else 0) for i in range(N_CHUNK)]
CO = [sum(CW[:i]) for i in range(N_CHUNK)]

_NC_CACHE = None


def _build_nc():
    """Build the SPMD Bass program (identical on all 8 cores)."""
    nc = bacc.Bacc(
        "TRN2", target_bir_lowering=False, debug=False, enable_partition_id=False
    )
    f32 = mybir.dt.float32
    x = nc.dram_tensor("x", [E_TOTAL, NP_USED, W], f32, kind="ExternalInput")
    out = nc.dram_tensor("out", [NP_USED, W], f32, kind="ExternalOutput")
    x_t = x.ap()
    out_t = out.ap()

    mult = AluOpType.mult
    add = AluOpType.add

    def load_plane(dst, e, lo, w):
        """Load DRAM rows [0,92) -> SBUF partitions [0,92) and DRAM rows
        [92,120) -> SBUF partitions [96,124), columns [lo, lo+w)."""
        nc.sync.dma_start(out=dst[0:92, 0:w], in_=x_t[e, 0:92, lo : lo + w])
        nc.sync.dma_start(out=dst[96:124, 0:w], in_=x_t[e, 92:120, lo : lo + w])

    with tile.TileContext(nc) as tc:
        with (
            tc.tile_pool(name="in", bufs=8) as pin,
            tc.tile_pool(name="acc", bufs=1) as pacc,
        ):
            acc = pacc.tile([PT, W], f32, name="acc", tag="acc")
            last = E_TOTAL - 1
            for e in range(E_TOTAL):
                if e < last:
                    t = pin.tile([PT, W], f32)
                    load_plane(t, e, 0, W)
                    if e == 0:
                        # acc = t0 * 1/3 (tensor_scalar: 2x perf mode)
                        nc.vector.tensor_scalar_mul(acc[:], t[:], SCALE)
                    else:
                        # acc = (t_e * 1/3) + acc
                        nc.vector.scalar_tensor_tensor(
                            acc[:], t[:], SCALE, acc[:], mult, add
                        )
                else:
                    # Last plane in four column-chunks: each chunk's final
                    # add fires on its own landing; its store follows on the
                    # ACT HWDGE ring (SP ring carries only loads).
                    for c in range(N_CHUNK):
                        ct = pin.tile([PT, CW[c]], f32, name=f"c{c}", tag=f"c{c}")
                        load_plane(ct, e, CO[c], CW[c])
                        cs = slice(CO[c], CO[c] + CW[c])
                        nc.vector.scalar_tensor_tensor(
                            acc[:, cs], ct[:], SCALE, acc[:, cs], mult, add
                        )
                        nc.scalar.dma_start(
                            out=out_t[0:92, cs], in_=acc[0:92, cs]
                        )
                        nc.scalar.dma_start(
                            out=out_t[92:120, cs], in_=acc[96:124, cs]
                        )
    nc.compile()
    return nc


def _get_nc():
    global _NC_CACHE
    if _NC_CACHE is None:
        _NC_CACHE = _build_nc()
    return _NC_CACHE


def _run(inputs, trace=False, trace_kwargs=None):
    e0 = np.asarray(inputs["expert_emb_0"], dtype=np.float32)
    e1 = np.asarray(inputs["expert_emb_1"], dtype=np.float32)
    e2 = np.asarray(inputs["expert_emb_2"], dtype=np.float32)
    B = e0.shape[1]
    assert B == N_CORES, f"expected B == {N_CORES}, got {B}"

    in_maps = []
    for b in range(B):
        xb = np.empty((E_TOTAL, NP_USED * W), dtype=np.float32)
        xb[:, PLANE:] = 0.0
        np.concatenate(
            [e0[:, b].reshape(-1, PLANE), e1[:, b].reshape(-1, PLANE),
             e2[:, b].reshape(-1, PLANE)],
            axis=0,
            out=xb[:, :PLANE],
        )
        in_maps.append({"x": xb.reshape(E_TOTAL, NP_USED, W)})

    kw = {}
    if trace:
        kw["trace"] = True
        if trace_kwargs:
            kw.update(trace_kwargs)
    try:
        res = run_bass_kernel_spmd(_get_nc(), in_maps, list(range(N_CORES)), **kw)
    except Exception:
        # One retry: transient device errors (e.g. NRT unrecoverable after a
        # prior wedged run) usually clear on re-dispatch.
        res = run_bass_kernel_spmd(_get_nc(), in_maps, list(range(N_CORES)), **kw)
    out = np.stack(
        [
            res.results[b]["out"].reshape(-1)[:PLANE].reshape(L, D)
            for b in range(B)
        ],
        axis=0,
    )
    return out.astype(np.float32, copy=False), res


def kernel(**inputs) -> np.ndarray:
    out, _ = _run(inputs, trace=False)
    return out


# revision 7
# speedup vs baseline: 1.1237x; 1.1237x over previous
"""DiffuseRouter kernel for 8 TRN2 NeuronCores.

Reference computation (enable_time=False, soft_time_routing=True):
    out[b, l, d] = (1/3) * sum_g sum_e expert_emb_g[e, b, l, d]
i.e. a uniform-weighted sum of 28 expert planes per batch element.

Sharding: pure data-parallel over batch B=8 -> one batch element per core.
Each core reads its 28 [256, 1280] f32 planes (36.7 MB), reduces them
on-chip with a DVE scalar_tensor_tensor chain (1/3 folded in), and writes
its [256, 1280] output.  No collectives needed (B == n_cores).

v5 -- dodge SDMA engine 15.  Profiled runs show engine 15 sustains only
~22 GB/s vs ~27 for engines 0-14 (runtime/profile traffic on its port),
and a plane's completion semaphore waits for the slowest engine, so the
whole pipeline paces at engine 15's rate.  Trace experiments show that
for *contiguous* DRAM sources the HWDGE deals row-descriptors to SDMA
engines round-robin starting at engine 0 per transfer -- so a transfer
with <= 15 descriptors never touches engine 15.  Each plane is therefore
loaded as 8x [15, 2560] + 1x [8, 2560] contiguous row-chunks (engines
0-7 get 9 descriptors per plane, 8-14 get 8, engine 15 none), into the
same [128, 2560] tile layout the DVE chain consumes.

The last plane is loaded in column-quarter chunks instead (per quarter,
still split 15/8 row-wise) so each final quarter-add -> store fires as
soon as its own ~0.33 MB lands, keeping the post-stream tail small.
"""

import numpy as np

import concourse.bacc as bacc
import concourse.tile as tile
from concourse import mybir
from concourse.alu_op_type import AluOpType
from concourse.bass_utils import run_bass_kernel_spmd

N_CORES = 8
E_TOTAL = 28  # 4 + 8 + 16 experts across the 3 granularity levels
L, D = 256, 1280
P = 128  # SBUF partitions
FD = (L // P) * D  # 2560 free-dim elements per partition
SCALE = 1.0 / 3.0

# Row-chunking: 8 chunks of 15 partitions + 1 of 8 (sum = 128), each chunk a
# contiguous DRAM range -> <=15 descriptors -> SDMA engines 0-14 only.
ROW_CHUNKS = [(15 * i, 15) for i in range(8)] + [(120, 8)]

_NC_CACHE = None


def _build_nc():
    nc = bacc.Bacc(
        "TRN2", target_bir_lowering=False, debug=False, enable_partition_id=False
    )
    f32 = mybir.dt.float32
    x = nc.dram_tensor("x", [E_TOTAL, L, D], f32, kind="ExternalInput")
    out = nc.dram_tensor("out", [L, D], f32, kind="ExternalOutput")

    # [E, 256, 1280] -> [E, 128, 2560]: partition p holds rows 2p, 2p+1
    # (contiguous 10240 B per partition).
    x_t = x.ap().rearrange("e (p a) d -> e p (a d)", a=2)
    out_t = out.ap().rearrange("(p a) d -> p (a d)", a=2)

    mult = AluOpType.mult
    add = AluOpType.add

    def load_rows(dst, e, lo, w):
        """Load columns [lo, lo+w) of plane e as 15/8-partition row chunks."""
        for r0, rn in ROW_CHUNKS:
            nc.sync.dma_start(
                out=dst[r0 : r0 + rn, 0:w], in_=x_t[e][r0 : r0 + rn, lo : lo + w]
            )

    H = FD // 2
    halves = [slice(0, H), slice(H, FD)]

    with tile.TileContext(nc) as tc:
        with (
            tc.tile_pool(name="in", bufs=8) as pin,
            tc.tile_pool(name="acc", bufs=1) as pacc,
        ):
            accs = [
                pacc.tile([P, H], f32, name=f"acc{i}", tag=f"acc{i}")
                for i in range(2)
            ]
            last = E_TOTAL - 1
            for e in range(E_TOTAL):
                if e < last:
                    t = pin.tile([P, FD], f32)
                    load_rows(t, e, 0, FD)
                    ths = [t[:, h] for h in halves]
                    for acc, th in zip(accs, ths):
                        if e == 0:
                            # acc = t0 * 1/3 (tensor_scalar: 2x perf mode)
                            nc.vector.tensor_scalar_mul(acc[:], th, SCALE)
                        else:
                            # acc = (t_e * 1/3) + acc
                            nc.vector.scalar_tensor_tensor(
                                acc[:], th, SCALE, acc[:], mult, add
                            )
                else:
                    # Last plane: four column-quarter loads so each final
                    # quarter-add fires on its own landing; store right after
                    # on the ACT HWDGE ring (SP ring carries only loads).
                    Q = FD // 4
                    for qi in range(4):
                        qt = pin.tile([P, Q], f32, name=f"tq{qi}", tag=f"tq{qi}")
                        load_rows(qt, e, qi * Q, Q)
                        acc = accs[qi // 2]
                        qs = slice((qi % 2) * Q, (qi % 2 + 1) * Q)
                        nc.vector.scalar_tensor_tensor(
                            acc[:, qs], qt[:], SCALE, acc[:, qs], mult, add
                        )
                        nc.scalar.dma_start(
                            out=out_t[:, qi * Q : (qi + 1) * Q], in_=acc[:, qs]
                        )
    nc.compile()
    return nc


def _get_nc():
    global _NC_CACHE
    if _NC_CACHE is None:
        _NC_CACHE = _build_nc()
    return _NC_CACHE


def _run(inputs, trace=False, trace_kwargs=None):
    e0 = np.asarray(inputs["expert_emb_0"], dtype=np.float32)
    e1 = np.asarray(inputs["expert_emb_1"], dtype=np.float32)
    e2 = np.asarray(inputs["expert_emb_2"], dtype=np.float32)
    B = e0.shape[1]
    assert B == N_CORES, f"expected B == {N_CORES}, got {B}"

    in_maps = []
    for b in range(B):
        xb = np.concatenate([e0[:, b], e1[:, b], e2[:, b]], axis=0)
        in_maps.append({"x": np.ascontiguousarray(xb)})

    kw = {}
    if trace:
        kw["trace"] = True
        if trace_kwargs:
            kw.update(trace_kwargs)
    try:
        res = run_bass_kernel_spmd(_get_nc(), in_maps, list(range(N_CORES)), **kw)
    except Exception:
        # One retry: transient device errors (e.g. NRT unrecoverable after a
        # prior wedged run) usually clear on re-dispatch.
        res = run_bass_kernel_spmd(_get_nc(), in_maps, list(range(N_CORES)), **kw)
    out = np.stack([res.results[b]["out"] for b in range(B)], axis=0)
    return out.astype(np.float32, copy=False), res


def kernel(**inputs) -> np.ndarray:
    out, _ = _run(inputs, trace=False)
    return out


# revision 10
# speedup vs baseline: 2.4634x; 2.1922x over previous
"""DiffuseRouter kernel for 8 TRN2 NeuronCores.

Reference computation (enable_time=False, soft_time_routing=True):
    out[b, l, d] = (1/3) * sum_g sum_e expert_emb_g[e, b, l, d]
i.e. a uniform-weighted sum of 28 expert planes per batch element.

Sharding: pure data-parallel over batch B=8 -> one batch element per core.
Each core reads its 28 expert planes (36.7 MB), reduces them on-chip with
a DVE scalar_tensor_tensor chain (1/3 folded in), and writes its
[256, 1280] output.  No collectives needed (B == n_cores).

v6 -- dodge SDMA engine 15 with host-packed [105, 3121] planes.
Profiled runs show SDMA engine 15 sustains only ~22 GB/s vs ~27 for
engines 0-14 (runtime/profile-stream traffic on its AXI port), and a
plane's completion semaphore waits for the slowest engine, so the whole
126-us pipeline paced at engine 15's rate.  Trace experiments established
two facts about the qSPDynamicHW ring: (1) for a contiguous DRAM source
the HWDGE splits it into SBUF-row descriptors and deals them to SDMA
engines round-robin, restarting at engine 0 for every transfer; (2) each
dma_start costs the ring ~0.6 us of descriptor-generation time, so a
plane must be few transfers.  Both facts together give the layout: pack
each plane as [105, 3121] f32 on the host (105 = 7*15 descriptors of
12,484 B -> engines 0..14 get exactly 7 descriptors each, engine 15
none) and load it with ONE dma_start.  Per-plane DMA time becomes
7*12,484 B / ~27 GB/s = 3.25 us on every live engine, ~91 us for the
stream, with no straggler.

The last plane (and the output) are host-packed as four contiguous
[105, 781] column-chunk tensors so each final quarter-add -> store fires
as soon as its own ~0.33 MB lands, keeping the post-stream tail to ~3 us.
"""

import numpy as np

import concourse.bacc as bacc
import concourse.tile as tile
from concourse import mybir
from concourse.alu_op_type import AluOpType
from concourse.bass_utils import run_bass_kernel_spmd

N_CORES = 8
E_TOTAL = 28  # 4 + 8 + 16 experts across the 3 granularity levels
L, D = 256, 1280
PLANE = L * D  # 327680 f32 per expert plane
NP = 105  # partitions used: 7*15 -> 7 descriptors on each of engines 0..14
NLAST = 4  # column-chunks of the last plane
CW = -(-PLANE // (NP * NLAST))  # 781: chunk width
W = NLAST * CW  # 3124 f32 per partition (340 pad zeros; all planes same map)
SCALE = 1.0 / 3.0

_NC_CACHE = None


def _build_nc():
    nc = bacc.Bacc(
        "TRN2", target_bir_lowering=False, debug=False, enable_partition_id=False
    )
    f32 = mybir.dt.float32
    x = nc.dram_tensor("x", [E_TOTAL - 1, NP, W], f32, kind="ExternalInput")
    xl = nc.dram_tensor("xl", [NLAST, NP, CW], f32, kind="ExternalInput")
    out = nc.dram_tensor("out", [NLAST, NP, CW], f32, kind="ExternalOutput")
    x_t = x.ap()
    xl_t = xl.ap()
    out_t = out.ap()

    mult = AluOpType.mult
    add = AluOpType.add

    with tile.TileContext(nc) as tc:
        with (
            tc.tile_pool(name="in", bufs=6) as pin,
            tc.tile_pool(name="acc", bufs=1) as pacc,
        ):
            acc = pacc.tile([NP, W], f32, name="acc", tag="acc")
            last = E_TOTAL - 1
            for e in range(E_TOTAL):
                if e < last:
                    t = pin.tile([NP, W], f32)
                    # One linear 1.31 MB load: 105 descriptors of 12,484 B,
                    # round-robined onto SDMA engines 0-14 (7 each).
                    nc.sync.dma_start(out=t[:], in_=x_t[e])
                    if e == 0:
                        # acc[:, :W] = t0 * 1/3 (tensor_scalar: 2x perf mode)
                        nc.vector.tensor_scalar_mul(acc[:], t[:], SCALE)
                    else:
                        # acc = (t_e * 1/3) + acc
                        nc.vector.scalar_tensor_tensor(
                            acc[:], t[:], SCALE, acc[:], mult, add
                        )
                else:
                    # Last plane: four contiguous [105, 781] chunk loads; each
                    # chunk's final add fires on its own landing, its store
                    # follows on the ACT HWDGE ring.
                    for c in range(NLAST):
                        ct = pin.tile([NP, CW], f32, name=f"c{c}", tag=f"c{c}")
                        nc.sync.dma_start(out=ct[:], in_=xl_t[c])
                        cs = slice(c * CW, (c + 1) * CW)
                        nc.vector.scalar_tensor_tensor(
                            acc[:, cs], ct[:], SCALE, acc[:, cs], mult, add
                        )
                        nc.scalar.dma_start(out=out_t[c], in_=acc[:, cs])
    nc.compile()
    return nc


def _get_nc():
    global _NC_CACHE
    if _NC_CACHE is None:
        _NC_CACHE = _build_nc()
    return _NC_CACHE


def _run(inputs, trace=False, trace_kwargs=None):
    e0 = np.asarray(inputs["expert_emb_0"], dtype=np.float32)
    e1 = np.asarray(inputs["expert_emb_1"], dtype=np.float32)
    e2 = np.asarray(inputs["expert_emb_2"], dtype=np.float32)
    B = e0.shape[1]
    assert B == N_CORES, f"expected B == {N_CORES}, got {B}"

    in_maps = []
    for b in range(B):
        planes = np.concatenate(
            [e0[:, b].reshape(-1, PLANE), e1[:, b].reshape(-1, PLANE),
             e2[:, b].reshape(-1, PLANE)],
            axis=0,
        )  # [28, 327680]
        # planes 0..26: pack to [105, 3124] each (340 trailing pad zeros)
        xb = np.zeros((E_TOTAL - 1, NP * W), dtype=np.float32)
        xb[:, :PLANE] = planes[: E_TOTAL - 1]
        # plane 27: four contiguous [105, 781] column-chunk tensors holding
        # columns [c*CW, (c+1)*CW) of the SAME [105, W] layout as the chain.
        lp = np.zeros((NP * NLAST * CW,), dtype=np.float32)
        lp[:PLANE] = planes[E_TOTAL - 1]
        lp = lp.reshape(NP, NLAST * CW)
        xlb = np.stack(
            [np.ascontiguousarray(lp[:, c * CW : (c + 1) * CW]) for c in range(NLAST)]
        )
        in_maps.append(
            {"x": xb.reshape(E_TOTAL - 1, NP, W), "xl": xlb}
        )

    kw = {}
    if trace:
        kw["trace"] = True
        if trace_kwargs:
            kw.update(trace_kwargs)
    try:
        res = run_bass_kernel_spmd(_get_nc(), in_maps, list(range(N_CORES)), **kw)
    except Exception:
        # One retry: transient device errors (e.g. NRT unrecoverable after a
        # prior wedged run) usually clear on re-dispatch.
        res = run_bass_kernel_spmd(_get_nc(), in_maps, list(range(N_CORES)), **kw)
    outs = []
    for b in range(B):
        o = res.results[b]["out"]  # [NLAST, NP, CW]
        flat = np.concatenate([o[c] for c in range(NLAST)], axis=1).reshape(-1)
        outs.append(flat[:PLANE].reshape(L, D))
    return np.stack(outs, axis=0).astype(np.float32, copy=False), res


def kernel(**inputs) -> np.ndarray:
    out, _ = _run(inputs, trace=False)
    return out


# revision 11
# speedup vs baseline: 2.7659x; 1.1228x over previous
"""DiffuseRouter kernel for 8 TRN2 NeuronCores.

Reference computation (enable_time=False, soft_time_routing=True):
    out[b, l, d] = (1/3) * sum_g sum_e expert_emb_g[e, b, l, d]
i.e. a uniform-weighted sum of 28 expert planes per batch element.

Sharding: pure data-parallel over batch B=8 -> one batch element per core.
Each core reads its 28 [256, 1280] f32 planes (36.7 MB), reduces them
on-chip, scales by 1/3, and writes its [256, 1280] output.  No collectives
needed (B == n_cores).

Engine assignment (v3): the DMA stream sustains ~425 GB/s aggregate
(plane completion paces at straggler SDMA engine 15, ~3.6 us/plane), so
the reduction is split across two engines that each keep pace:

  * TensorE sums free-dim columns [0, 1536) via identity matmuls
    accumulating into 3 PSUM banks (fp32r moving operand, 1 cycle/row;
    fp32r never leaves the 1.2 GHz MID clock, so a full 5-bank PE
    version at ~3.9 us/plane would throttle the stream -- 3 banks run
    at ~2.4 us/plane).  ACT applies the final x1/3 from PSUM per bank.
  * DVE sums columns [1536, 2560) with a scalar_tensor_tensor chain
    (fp32 1x mode, ~1.2 us/plane) with the 1/3 scale folded in.

Only the natural [128, 2560] full-partition contiguous plane transfer
runs the SDMA engines at line rate (~27 GB/s each); every partial or
repacked shape measured 20-30% slower (engine/port misalignment), so
engine 15's ~22 GB/s is accepted as the pacing floor.

The last plane is loaded as bank/column chunks (PE's chunks first, the
DVE chunks after, a small 256-col final chunk) so each column range's
final op -> store fires as soon as its own chunk lands, keeping the
post-stream tail to ~2-3 us.
"""

import numpy as np

import concourse.bacc as bacc
import concourse.tile as tile
from concourse import mybir
from concourse.alu_op_type import AluOpType
from concourse.bass_utils import run_bass_kernel_spmd

N_CORES = 8
E_TOTAL = 28  # 4 + 8 + 16 experts across the 3 granularity levels
L, D = 256, 1280
P = 128  # SBUF partitions
FD = (L // P) * D  # 2560 free-dim elements per partition
BW = 512  # one 2 KB PSUM bank of f32
NB_PE = 3  # banks summed on TensorE (cols 0..1536)
DVE_LO = NB_PE * BW  # 1536: start of the DVE column range
DVE_W = FD - DVE_LO  # 1024 cols summed on DVE
SCALE = 1.0 / 3.0

_NC_CACHE = None


def _build_nc():
    """Build the SPMD Bass program (identical on all 8 cores)."""
    nc = bacc.Bacc(
        "TRN2", target_bir_lowering=False, debug=False, enable_partition_id=False
    )
    f32 = mybir.dt.float32
    f32r = mybir.dt.float32r
    x = nc.dram_tensor("x", [E_TOTAL, L, D], f32, kind="ExternalInput")
    ident_d = nc.dram_tensor("ident", [P, P], f32, kind="ExternalInput")
    out = nc.dram_tensor("out", [L, D], f32, kind="ExternalOutput")

    # [E, 256, 1280] -> [E, 128, 2560]: partition p holds rows 2p, 2p+1
    # (contiguous 10240 B per partition -> fully linear 1.31 MB DMA per plane).
    x_t = x.ap().rearrange("e (p a) d -> e p (a d)", a=2)
    x_tr = x_t.bitcast(f32r)
    out_t = out.ap().rearrange("(p a) d -> p (a d)", a=2)

    mult = AluOpType.mult
    add = AluOpType.add

    with tile.TileContext(nc) as tc:
        with (
            tc.tile_pool(name="in", bufs=8) as pin,
            tc.tile_pool(name="const", bufs=1) as pconst,
            tc.tile_pool(name="acc", bufs=1) as pacc,
            tc.tile_pool(name="ps", bufs=1, space="PSUM") as pps,
        ):
            ident = pconst.tile([P, P], f32r, name="ident", tag="ident")
            # Identity comes in from DRAM on the ACT ring so the SP ring
            # carries nothing but the 28 plane loads.
            nc.scalar.dma_start(out=ident[:], in_=ident_d.ap().bitcast(f32r))
            psums = [
                pps.tile([P, BW], f32, name=f"ps{b}", tag=f"ps{b}")
                for b in range(NB_PE)
            ]
            # ACT staging for the PE banks' scaled output.
            outs = pacc.tile([P, NB_PE * BW], f32, name="outs", tag="outs")
            # DVE accumulator for cols [1536, 2560), scale folded into adds.
            acc = pacc.tile([P, DVE_W], f32, name="acc", tag="acc")

            last = E_TOTAL - 1
            for e in range(E_TOTAL):
                if e < last:
                    # One linear 1.31 MB load per plane; PE reads the f32r
                    # view, DVE reads the same bytes bitcast back to f32.
                    t = pin.tile([P, FD], f32r)
                    nc.sync.dma_start(out=t[:], in_=x_tr[e])
                    pe_chunks = [t[:, b * BW : (b + 1) * BW] for b in range(NB_PE)]
                    dve_chunks = [(DVE_LO, DVE_W, t[:, DVE_LO:FD].bitcast(f32))]
                else:
                    # Last plane: bank/column chunk loads in separate tiles so
                    # each column range's final op starts as soon as its own
                    # chunk lands.  PE chunks load first; the DVE range loads
                    # as a 768-col chunk then a small 256-col final chunk so
                    # the very last add+store is short.
                    pe_chunks = []
                    for b in range(NB_PE):
                        ct = pin.tile([P, BW], f32r, name=f"c{b}", tag=f"c{b}")
                        nc.sync.dma_start(
                            out=ct[:], in_=x_tr[e][:, b * BW : (b + 1) * BW]
                        )
                        pe_chunks.append(ct[:])
                    dve_chunks = []
                    for lo, w in ((DVE_LO, 768), (DVE_LO + 768, 256)):
                        ct = pin.tile([P, w], f32, name=f"d{lo}", tag=f"d{lo}")
                        nc.sync.dma_start(out=ct[:], in_=x_t[e][:, lo : lo + w])
                        dve_chunks.append((lo, w, ct[:]))

                for b in range(NB_PE):
                    # psum[b] (+)= chunk  via  I.T @ chunk, fp32r single-pass.
                    nc.tensor.matmul(
                        psums[b][:],
                        ident[:],
                        pe_chunks[b],
                        start=(e == 0),
                        stop=(e == last),
                    )
                    if e == last:
                        bs = slice(b * BW, (b + 1) * BW)
                        # ACT: out = psum * 1/3 (PSUM -> SBUF), then store on
                        # the ACT HWDGE ring (SP ring is busy with loads).
                        nc.scalar.mul(outs[:, bs], psums[b][:], SCALE)
                        nc.scalar.dma_start(out=out_t[:, bs], in_=outs[:, bs])

                for lo, w, th in dve_chunks:
                    qs = slice(lo - DVE_LO, lo - DVE_LO + w)
                    if e == 0:
                        # acc = t0 * 1/3 (tensor_scalar: 2x perf mode)
                        nc.vector.tensor_scalar_mul(acc[:, qs], th, SCALE)
                    else:
                        # acc = (t_e * 1/3) + acc
                        nc.vector.scalar_tensor_tensor(
                            acc[:, qs], th, SCALE, acc[:, qs], mult, add
                        )
                    if e == last:
                        nc.scalar.dma_start(
                            out=out_t[:, lo : lo + w], in_=acc[:, qs]
                        )
    nc.compile()
    return nc


def _get_nc():
    global _NC_CACHE
    if _NC_CACHE is None:
        _NC_CACHE = _build_nc()
    return _NC_CACHE


def _run(inputs, trace=False, trace_kwargs=None):
    e0 = np.asarray(inputs["expert_emb_0"], dtype=np.float32)
    e1 = np.asarray(inputs["expert_emb_1"], dtype=np.float32)
    e2 = np.asarray(inputs["expert_emb_2"], dtype=np.float32)
    B = e0.shape[1]
    assert B == N_CORES, f"expected B == {N_CORES}, got {B}"

    ident = np.eye(P, dtype=np.float32)
    in_maps = []
    for b in range(B):
        xb = np.concatenate([e0[:, b], e1[:, b], e2[:, b]], axis=0)
        in_maps.append({"x": np.ascontiguousarray(xb), "ident": ident})

    kw = {}
    if trace:
        kw["trace"] = True
        if trace_kwargs:
            kw.update(trace_kwargs)
    try:
        res = run_bass_kernel_spmd(_get_nc(), in_maps, list(range(N_CORES)), **kw)
    except Exception:
        # One retry: transient device errors (e.g. NRT unrecoverable after a
        # prior wedged run) usually clear on re-dispatch.
        res = run_bass_kernel_spmd(_get_nc(), in_maps, list(range(N_CORES)), **kw)
    out = np.stack([res.results[b]["out"] for b in range(B)], axis=0)
    return out.astype(np.float32, copy=False), res


def kernel(**inputs) -> np.ndarray:
    out, _ = _run(inputs, trace=False)
    return out


# revision 20
# speedup vs baseline: 3.3284x; 1.2034x over previous
"""DiffuseRouter kernel for 8 TRN2 NeuronCores.

Reference computation (enable_time=False, soft_time_routing=True):
    out[b, l, d] = (1/3) * sum_g sum_e expert_emb_g[e, b, l, d]
i.e. a uniform-weighted sum of 28 expert planes per batch element.

Sharding: pure data-parallel over batch B=8 -> one batch element per core.
Each core reads its 28 [256, 1280] f32 planes (36.7 MB), reduces them
on-chip, scales by 1/3, and writes its [256, 1280] output.  No collectives
needed (B == n_cores).

Engine assignment (v3): the DMA stream sustains ~425 GB/s aggregate
(plane completion paces at straggler SDMA engine 15, ~3.6 us/plane), so
the reduction is split across two engines that each keep pace:

  * TensorE sums free-dim columns [0, 1536) via identity matmuls
    accumulating into 3 PSUM banks (fp32r moving operand, 1 cycle/row;
    fp32r never leaves the 1.2 GHz MID clock, so a full 5-bank PE
    version at ~3.9 us/plane would throttle the stream -- 3 banks run
    at ~2.4 us/plane).  ACT applies the final x1/3 from PSUM per bank.
  * DVE sums columns [1536, 2560) with a scalar_tensor_tensor chain
    (fp32 1x mode, ~1.2 us/plane) with the 1/3 scale folded in.

Only the natural [128, 2560] full-partition contiguous plane transfer
runs the SDMA engines at line rate (~27 GB/s each); every partial or
repacked shape measured 20-30% slower (engine/port misalignment), so
engine 15's ~22 GB/s is accepted as the pacing floor.

The last plane is loaded as bank/column chunks (PE's chunks first, the
DVE chunks after, a small 256-col final chunk) so each column range's
final op -> store fires as soon as its own chunk lands, keeping the
post-stream tail to ~2-3 us.
"""

import numpy as np

import concourse.bacc as bacc
import concourse.tile as tile
from concourse import mybir
from concourse.alu_op_type import AluOpType
from concourse.bass_utils import run_bass_kernel_spmd

N_CORES = 8
E_TOTAL = 28  # 4 + 8 + 16 experts across the 3 granularity levels
L, D = 256, 1280
P = 128  # SBUF partitions
FD = (L // P) * D  # 2560 free-dim elements per partition
BW = 512  # one 2 KB PSUM bank of f32
NB_PE = 3  # banks summed on TensorE (cols 0..1536)
DVE_LO = NB_PE * BW  # 1536: start of the DVE column range
DVE_W = FD - DVE_LO  # 1024 cols summed on DVE
SCALE = 1.0 / 3.0

_NC_CACHE = None


def _build_nc():
    """Build the SPMD Bass program (identical on all 8 cores)."""
    nc = bacc.Bacc(
        "TRN2", target_bir_lowering=False, debug=False, enable_partition_id=False
    )
    f32 = mybir.dt.float32
    f32r = mybir.dt.float32r
    x = nc.dram_tensor("x", [E_TOTAL, L, D], f32, kind="ExternalInput")
    ident_d = nc.dram_tensor("ident", [P, P], f32, kind="ExternalInput")
    out = nc.dram_tensor("out", [L, D], f32, kind="ExternalOutput")

    # [E, 256, 1280] -> [E, 128, 2560]: partition p holds rows 2p, 2p+1
    # (contiguous 10240 B per partition -> fully linear 1.31 MB DMA per plane).
    x_t = x.ap().rearrange("e (p a) d -> e p (a d)", a=2)
    x_tr = x_t.bitcast(f32r)
    out_t = out.ap().rearrange("(p a) d -> p (a d)", a=2)

    mult = AluOpType.mult
    add = AluOpType.add

    with tile.TileContext(nc) as tc:
        with (
            tc.tile_pool(name="in", bufs=8) as pin,
            tc.tile_pool(name="const", bufs=1) as pconst,
            tc.tile_pool(name="acc", bufs=1) as pacc,
            tc.tile_pool(name="ps", bufs=1, space="PSUM") as pps,
        ):
            ident = pconst.tile([P, P], f32r, name="ident", tag="ident")
            # Identity comes in from DRAM on the ACT ring so the SP ring
            # carries nothing but the 28 plane loads.
            nc.scalar.dma_start(out=ident[:], in_=ident_d.ap().bitcast(f32r))
            psums = [
                pps.tile([P, BW], f32, name=f"ps{b}", tag=f"ps{b}")
                for b in range(NB_PE)
            ]
            # ACT staging for the PE banks' scaled output.
            outs = pacc.tile([P, NB_PE * BW], f32, name="outs", tag="outs")
            # DVE accumulator for cols [1536, 2560), scale folded into adds.
            acc = pacc.tile([P, DVE_W], f32, name="acc", tag="acc")

            last = E_TOTAL - 1
            for e in range(E_TOTAL):
                if e < last:
                    # One linear 1.31 MB load per plane; PE reads the f32r
                    # view, DVE reads the same bytes bitcast back to f32.
                    t = pin.tile([P, FD], f32r)
                    nc.sync.dma_start(out=t[:], in_=x_tr[e])
                    pe_chunks = [t[:, b * BW : (b + 1) * BW] for b in range(NB_PE)]
                    dve_chunks = [(DVE_LO, DVE_W, t[:, DVE_LO:FD].bitcast(f32))]
                else:
                    # Last plane: bank/column chunk loads in separate tiles so
                    # each column range's final op starts as soon as its own
                    # chunk lands.  PE chunks load first; the DVE range loads
                    # as a 768-col chunk then a small 256-col final chunk so
                    # the very last add+store is short.
                    pe_chunks = []
                    for b in range(NB_PE):
                        ct = pin.tile([P, BW], f32r, name=f"c{b}", tag=f"c{b}")
                        nc.sync.dma_start(
                            out=ct[:], in_=x_tr[e][:, b * BW : (b + 1) * BW]
                        )
                        pe_chunks.append(ct[:])
                    dve_chunks = []
                    for lo, w in ((DVE_LO, 768), (DVE_LO + 768, 256)):
                        ct = pin.tile([P, w], f32, name=f"d{lo}", tag=f"d{lo}")
                        nc.sync.dma_start(out=ct[:], in_=x_t[e][:, lo : lo + w])
                        dve_chunks.append((lo, w, ct[:]))

                for b in range(NB_PE):
                    # psum[b] (+)= chunk  via  I.T @ chunk, fp32r single-pass.
                    nc.tensor.matmul(
                        psums[b][:],
                        ident[:],
                        pe_chunks[b],
                        start=(e == 0),
                        stop=(e == last),
                    )
                    if e == last:
                        bs = slice(b * BW, (b + 1) * BW)
                        # ACT: out = psum * 1/3 (PSUM -> SBUF), then store on
                        # the ACT HWDGE ring (SP ring is busy with loads).
                        nc.scalar.mul(outs[:, bs], psums[b][:], SCALE)
                        nc.scalar.dma_start(out=out_t[:, bs], in_=outs[:, bs])

                for lo, w, th in dve_chunks:
                    qs = slice(lo - DVE_LO, lo - DVE_LO + w)
                    if e == 0:
                        # acc = t0 * 1/3 (tensor_scalar: 2x perf mode)
                        nc.vector.tensor_scalar_mul(acc[:, qs], th, SCALE)
                    else:
                        # acc = (t_e * 1/3) + acc
                        nc.vector.scalar_tensor_tensor(
                            acc[:, qs], th, SCALE, acc[:, qs], mult, add
                        )
                    if e == last:
                        nc.scalar.dma_start(
                            out=out_t[:, lo : lo + w], in_=acc[:, qs]
                        )
    nc.compile()
    return nc


def _get_nc():
    global _NC_CACHE
    if _NC_CACHE is None:
        _NC_CACHE = _build_nc()
    return _NC_CACHE


def _run(inputs, trace=False, trace_kwargs=None):
    e0 = np.asarray(inputs["expert_emb_0"], dtype=np.float32)
    e1 = np.asarray(inputs["expert_emb_1"], dtype=np.float32)
    e2 = np.asarray(inputs["expert_emb_2"], dtype=np.float32)
    B = e0.shape[1]
    assert B == N_CORES, f"expected B == {N_CORES}, got {B}"

    ident = np.eye(P, dtype=np.float32)
    in_maps = []
    for b in range(B):
        xb = np.concatenate([e0[:, b], e1[:, b], e2[:, b]], axis=0)
        in_maps.append({"x": np.ascontiguousarray(xb), "ident": ident})

    kw = {}
    if trace:
        kw["trace"] = True
        if trace_kwargs:
            kw.update(trace_kwargs)
    try:
        res = run_bass_kernel_spmd(_get_nc(), in_maps, list(range(N_CORES)), **kw)
    except Exception:
        # One retry: transient device errors (e.g. NRT unrecoverable after a
        # prior wedged run) usually clear on re-dispatch.
        res = run_bass_kernel_spmd(_get_nc(), in_maps, list(range(N_CORES)), **kw)
    out = np.stack([res.results[b]["out"] for b in range(B)], axis=0)
    return out.astype(np.float32, copy=False), res


def kernel(**inputs) -> np.ndarray:
    out, _ = _run(inputs, trace=False)
    return out
